# revision 14
# baseline (speedup 1.0000x reference)
"""DTW loss on trn2 — skewed-wavefront 2-core kernel, raw-bass phase B.

Same algorithm as kernel.py (skewed barber-pole wavefront, PE V-shift),
but emitted without TileContext: engines synchronize via manual counting
semaphores, so same-engine dependent ops run back-to-back without the
~95ns semaphore-propagation gap Tile inserts on every RAW hop.
"""

import sys

sys.path.insert(0, "/opt/trn_rl_repo")

from contextlib import ExitStack

import numpy as np

N = 4096
M = 4096
DIM = 64
N_ROWS = N // 2
F = 32
S = M // F
ROWS_PAD = N_ROWS + 256
NSTEP = N_ROWS + S - 1    # 2175
BIG = 1e30
ND = 12                   # dext rotation depth (capture WAR slack)
NCB = 6                   # cbuf rotation depth (cost-tile prefetch slack)

_nc_cache = {}


def _build_nc():
    if "nc" in _nc_cache:
        return _nc_cache["nc"]
    import concourse.bacc as bacc
    import concourse.bass as bass
    import concourse.mybir as mybir

    F32 = mybir.dt.float32
    BF16 = mybir.dt.bfloat16
    AluOp = mybir.AluOpType
    n_bands = N_ROWS // 128        # 16
    ROW_PITCH = 3 * M
    NB = 3 * n_bands               # 48 band-passes
    NMM = 8 * NB                   # 384 matmuls
    TB = 8
    NBATCH = (NSTEP + TB - 1) // TB

    nc = bacc.Bacc(None, target_bir_lowering=False)

    xt_d = nc.dram_tensor("xt", [65, N_ROWS], BF16, kind="ExternalInput")
    xst_d = nc.dram_tensor("xst", [65, N_ROWS], BF16, kind="ExternalInput")
    yt_d = nc.dram_tensor("yt", [65, M], BF16, kind="ExternalInput")
    yst_d = nc.dram_tensor("yst", [65, M], BF16, kind="ExternalInput")
    xx_d = nc.dram_tensor("xx", [128, N_ROWS // 128], F32, kind="ExternalInput")
    xxs_d = nc.dram_tensor("xxs", [128, N_ROWS // 128], F32, kind="ExternalInput")
    sh_d = nc.dram_tensor("sh", [128, 128], F32, kind="ExternalInput")
    m0_d = nc.dram_tensor("m0", [128, 1], F32, kind="ExternalInput")
    dinit_d = nc.dram_tensor("dinit", [128, 33], F32, kind="ExternalInput")
    out_d = nc.dram_tensor("out", [M], F32, kind="ExternalOutput")
    SI = nc.dram_tensor("SI", [ROWS_PAD * ROW_PITCH], BF16)

    es = ExitStack()

    def sb(nm, shape, dt):
        return es.enter_context(nc.sbuf_tensor(nm, shape, dt)).ap()

    def psb(nm, shape):
        return es.enter_context(nc.psum_tensor(nm, shape, F32)).ap()

    xt_sb = sb("xt_sb", [65, N_ROWS], BF16)
    xst_sb = sb("xst_sb", [65, N_ROWS], BF16)
    yt_sb = sb("yt_sb", [65, M], BF16)
    yst_sb = sb("yst_sb", [65, M], BF16)
    xx_sb = sb("xx_sb", [128, n_bands], F32)
    xxs_sb = sb("xxs_sb", [128, n_bands], F32)
    sh_sb = sb("sh_sb", [128, 128], F32)
    m0 = sb("m0_sb", [128, 1], F32)
    dext = [sb(f"dext{i}", [128, F + 1], F32) for i in range(ND)]
    zpad = sb("zpad", [128, 512], BF16)
    c_band = [sb(f"c_band{i}", [128, M], BF16) for i in range(4)]
    cbuf = [sb(f"cbuf{i}", [128, 3 * TB * F], BF16) for i in range(NCB)]
    bigrow = sb("bigrow", [1, 128], F32)
    onec = sb("onec", [1, 1], F32)
    t1s = sb("t1s", [128, F], F32)
    t2s = sb("t2s", [128, F], F32)
    ps_a = [psb(f"ps_a{i}", [128, 512]) for i in range(3)]
    vps = [psb(f"vps{i}", [128, 1]) for i in range(4)]

    s_in = nc.alloc_semaphore("s_in")
    s_mm = nc.alloc_semaphore("s_mm")
    s_act = nc.alloc_semaphore("s_act")
    s_store = nc.alloc_semaphore("s_store")
    s_pad = nc.alloc_semaphore("s_pad")
    s_cload = nc.alloc_semaphore("s_cload")
    s_dve = nc.alloc_semaphore("s_dve")
    s_shift = nc.alloc_semaphore("s_shift")
    s_cap = nc.alloc_semaphore("s_cap")

    IN_TOTAL = 9 * 16 + 2

    passes = (
        (xst_sb, xxs_sb, yt_sb),
        (xst_sb, xxs_sb, yst_sb),
        (xt_sb, xx_sb, yst_sb),
    )

    # ---------------- Pool: zpad memset + pad stores + captures ------
    # (keeps SP free for input loads + cbuf gathers; SP saturates at
    # ~650ns of sequencer time per dma_start)
    nc.gpsimd.memset(zpad, 0.0).then_inc(s_in, 1)
    nc.gpsimd.memset(bigrow, 0.0)
    nc.gpsimd.memset(bigrow[0:1, 127:128], BIG)
    nc.gpsimd.memset(onec, 1.0).then_inc(s_in, 1)
    # region-0 pads (SI rows 0..127, read from batch 0) first, then
    # region-1 (rows beyond the last real row, read from batch ~256)
    for r0 in (0, 128 + N_ROWS):
        for q in range(3):
            for ch in range(M // 512):
                dst = bass.AP(
                    SI[:].tensor, r0 * ROW_PITCH + q * M + ch * 512,
                    [[ROW_PITCH, 128], [1, 512]],
                )
                nc.gpsimd.dma_start(dst, zpad).then_inc(s_pad, 16)
    PAD_HALF = 16 * 3 * (M // 512)
    # captures (emitted below, interleaved per batch) follow on this queue

    # ---------------- SP queue ----------------
    nc.sync.dma_start(xt_sb, xt_d[:]).then_inc(s_in, 16)
    nc.sync.dma_start(xst_sb, xst_d[:]).then_inc(s_in, 16)
    nc.sync.dma_start(yt_sb, yt_d[:]).then_inc(s_in, 16)
    nc.sync.dma_start(yst_sb, yst_d[:]).then_inc(s_in, 16)
    nc.sync.dma_start(xx_sb, xx_d[:]).then_inc(s_in, 16)
    nc.sync.dma_start(xxs_sb, xxs_d[:]).then_inc(s_in, 16)
    nc.sync.dma_start(sh_sb, sh_d[:]).then_inc(s_in, 16)
    nc.sync.dma_start(m0, m0_d[:]).then_inc(s_in, 16)
    nc.sync.dma_start(dext[ND - 1], dinit_d[:]).then_inc(s_in, 16)

    pending_caps = []
    for k in range(NBATCH):
        t0 = k * TB
        band_needed = min(n_bands - 1, (t0 + TB - 1) // 128)
        nc.sync.wait_ge(s_store, 16 * 3 * (band_needed + 1))
        if k >= NCB:
            nc.sync.wait_ge(s_dve, 8 * (k - NCB) + 8)
        if k == 0:
            nc.sync.wait_ge(s_pad, PAD_HALF)
        if k == 256:
            nc.sync.wait_ge(s_pad, 2 * PAD_HALF)
        for q in range(3):
            src = bass.AP(
                SI[:].tensor,
                (t0 + 1) * ROW_PITCH + q * M + 127 * F,
                [[ROW_PITCH - F, 128], [ROW_PITCH, TB], [1, F]],
            )
            nc.sync.dma_start(
                cbuf[k % NCB][:, q * TB * F:(q + 1) * TB * F], src
            ).then_inc(s_cload, 16)
        # captures ride the Pool queue so they never head-of-line block
        # the SP cbuf gathers
        for t in pending_caps:
            s_c = t - (N_ROWS - 1)
            nc.gpsimd.wait_ge(s_dve, t + 1)
            nc.gpsimd.dma_start(
                out_d[s_c * F:(s_c + 1) * F],
                dext[t % ND][127 - s_c:128 - s_c, 1:F + 1],
            ).then_inc(s_cap, 16)
        pending_caps = [
            t for t in range(t0, min(t0 + TB, NSTEP))
            if 0 <= t - (N_ROWS - 1) < S
        ]
    for t in pending_caps:
        s_c = t - (N_ROWS - 1)
        nc.gpsimd.wait_ge(s_dve, t + 1)
        nc.gpsimd.dma_start(
            out_d[s_c * F:(s_c + 1) * F],
            dext[t % ND][127 - s_c:128 - s_c, 1:F + 1],
        ).then_inc(s_cap, 16)
    nc.sync.wait_ge(s_cap, 16 * S)
    _epilogue_sems = True  # placed after all queues are emitted (see below)

    # ---------------- ACT queue ----------------
    nc.scalar.wait_ge(s_in, IN_TOTAL)
    for u in range(NB):
        b, q = divmod(u, 3)
        bias_sb = passes[q][1]
        cb_t = c_band[u % 4]
        if u >= 4:
            nc.scalar.wait_ge(s_store, 16 * (u - 3))
        for i in range(8):
            nc.scalar.wait_ge(s_mm, 8 * u + i + 1)
            nc.scalar.activation(
                cb_t[:, i * 512:(i + 1) * 512],
                ps_a[(8 * u + i) % 3],
                mybir.ActivationFunctionType.Sqrt,
                bias=bias_sb[:, b:b + 1],
                scale=-2.0,
            ).then_inc(s_act, 1)
        dst = bass.AP(
            SI[:].tensor, (128 + b * 128) * ROW_PITCH + q * M,
            [[ROW_PITCH, 128], [1, M]],
        )
        nc.scalar.dma_start(dst, cb_t).then_inc(s_store, 16)

    # ---------------- PE queue ----------------
    def emit_mm(j):
        u, i = divmod(j, 8)
        b, q = divmod(u, 3)
        lhs, _, rhs = passes[q]
        if j >= 3:
            nc.tensor.wait_ge(s_act, j - 2)
        nc.tensor.matmul(
            ps_a[j % 3],
            lhs[:, b * 128:(b + 1) * 128],
            rhs[:, i * 512:(i + 1) * 512],
        ).then_inc(s_mm, 1)

    nc.tensor.wait_ge(s_in, IN_TOTAL)
    for j in range(2 * 24):          # bands 0,1 upfront
        emit_mm(j)
    for t in range(NSTEP):
        if t >= 1:
            nc.tensor.wait_ge(s_dve, t)
        nc.tensor.matmul(
            vps[t % 4], sh_sb, dext[(t - 1) % ND][:, F:F + 1],
            start=True, stop=False,
        )
        nc.tensor.matmul(
            vps[t % 4], bigrow, onec, start=False, stop=True,
        ).then_inc(s_shift, 1)
        # drip-feed band (t//128 + 2)'s matmuls
        b = t // 128 + 2
        if b < n_bands:
            lt = t % 128
            if lt % 5 == 0 and lt // 5 < 24:
                emit_mm(24 * b + lt // 5)

    # ---------------- DVE queue ----------------
    # NOTE: no engine-local ordering sem on the cross-step RAW edge —
    # same-engine program order covers it in this execution environment
    # (bass2jax/PJRT in-order queues); saves ~35ns/step of sem latency.
    nc.vector.wait_ge(s_in, IN_TOTAL)
    for t in range(NSTEP):
        k, dt = divmod(t, TB)
        if dt == 0:
            nc.vector.wait_ge(s_cload, 16 * 3 * (k + 1))
        cb = cbuf[k % NCB]
        a_t = cb[:, dt * F:(dt + 1) * F]
        b2_t = cb[:, (TB + dt) * F:(TB + dt + 1) * F]
        g_t = cb[:, (2 * TB + dt) * F:(2 * TB + dt + 1) * F]
        dcur = dext[t % ND]
        dprev = dext[(t - 1) % ND]
        nc.vector.tensor_tensor(t1s, dprev[:, 1:F + 1], a_t, op=AluOp.add)
        nc.vector.scalar_tensor_tensor(
            t2s, b2_t, 2.0, dprev[:, 0:F], op0=AluOp.mult, op1=AluOp.add
        )
        nc.vector.tensor_tensor(t2s, t1s, t2s, op=AluOp.min)
        nc.vector.wait_ge(s_shift, t + 1)
        if t >= N_ROWS - 1 + ND:
            # capture WAR: dext[t%ND] holds a captured row until its DMA done
            nc.vector.wait_ge(s_cap, 16 * (t - ND - (N_ROWS - 1) + 1))
        nc.vector.tensor_tensor_scan(
            dcur[:, 1:F + 1], g_t, t2s, vps[t % 4][:, 0:1],
            op0=AluOp.add, op1=AluOp.min,
        )
        # halo (next step's t2 shifted-D col 0) — off the critical path
        nc.vector.tensor_copy(dcur[:, 0:1], vps[t % 4][:, 0:1]).then_inc(
            s_dve, 1
        )

    # Tile-style epilogue: barrier + sem clear so the program is
    # re-executable (the runtime may run the NEFF more than once; stale
    # semaphore values would instantly satisfy every wait on rerun).
    nc.all_engine_barrier()
    nc.clear_and_free_semaphores(
        [s_in, s_mm, s_act, s_store, s_pad, s_cload, s_dve, s_shift, s_cap]
    )
    nc.all_engine_barrier()

    nc.compile()
    _nc_cache["nc"] = nc
    _nc_cache["es"] = es
    return nc


def _aug_t(a):
    """[n, 64] -> [65, n] transposed + ones row (bf16)."""
    import ml_dtypes
    n = a.shape[0]
    t = np.ones((65, n), np.float32)
    t[:64] = a.T
    return t.astype(ml_dtypes.bfloat16)


def _aug_y(yf):
    """[m, 64] -> [65, m]: y^T with row 64 = -0.5*(|y|^2 + 1e-12) (bf16)."""
    import ml_dtypes
    t = np.empty((65, yf.shape[0]), np.float32)
    t[:64] = yf.T
    yy = (yf.astype(np.float64) ** 2).sum(1)
    t[64] = (-0.5 * (yy + 1e-12)).astype(np.float32)
    return t.astype(ml_dtypes.bfloat16)


def _shift_mat():
    # partition p owns seg 127-p; V for seg s comes from seg s-1, i.e.
    # partition p+1: out[p] = rhs[p+1] -> lhsT[k,p]=1 iff k=p+1
    sh = np.zeros((128, 128), np.float32)
    for p in range(0, 127):
        sh[p + 1, p] = 1.0
    return sh


def _host_prep(xh, yf, core):
    if core == 0:
        xs, ys = xh, yf
    else:
        xs = np.concatenate([np.zeros((1, DIM), np.float32), xh[:-1]])
        # ys[0] pairs with x[0] to make C3[0,0] ~ 0 (H[0,0]=0). Perturb it
        # so the fp32-cancelled squared distance stays safely positive;
        # the 1.0 offset costs ~1e-5 relative error on the final answer.
        y0 = xh[0:1].copy()
        y0[0, 0] += 1.0
        ys = np.concatenate([y0, yf[:-1]])
    xx = (xh.astype(np.float64) ** 2).sum(1).astype(np.float32)
    xxs = (xs.astype(np.float64) ** 2).sum(1).astype(np.float32)
    return {
        "xt": _aug_t(xh),
        "xst": _aug_t(xs),
        "yt": _aug_y(yf),
        "yst": _aug_y(ys),
        # [p, b] = value for row b*128+p (contiguous [128, n_bands] load)
        "xx": np.ascontiguousarray(xx.reshape(-1, 128).T),
        "xxs": np.ascontiguousarray(xxs.reshape(-1, 128).T),
        "sh": _shift_mat(),
        "m0": _m0_mask(),
        "dinit": _dinit(),
    }


def _m0_mask():
    m = np.full((128, 1), -BIG, np.float32)
    m[127, 0] = BIG  # partition 127 = segment 0: V is always BIG there
    return m


def _dinit():
    d = np.full((128, 33), BIG, np.float32)
    d[127, 1] = 0.0  # virtual row -1: D(-1, global col 0) = 0
    return d




def kernel(x, y):
    x = np.ascontiguousarray(np.asarray(x, dtype=np.float32))
    y = np.ascontiguousarray(np.asarray(y, dtype=np.float32))
    assert x.shape == (N, DIM) and y.shape == (M, DIM)

    from concourse.bass_utils import run_bass_kernel_spmd

    nc = _build_nc()
    in_maps = [
        _host_prep(x[:N_ROWS], y, core=0),
        _host_prep(x[::-1][:N_ROWS].copy(), y[::-1].copy(), core=1),
    ]
    res = run_bass_kernel_spmd(nc, in_maps, core_ids=[0, 1])
    F_last = res.results[0]["out"].astype(np.float64)
    H_last = res.results[1]["out"].astype(np.float64)

    xm = x[N_ROWS].astype(np.float64)
    sq = (xm * xm).sum() + (y.astype(np.float64) ** 2).sum(1) - 2.0 * (
        y.astype(np.float64) @ xm
    )
    c_mid = np.sqrt(np.maximum(sq, 1e-12))
    B_row = H_last[::-1]
    cand_v = F_last + c_mid + B_row
    cand_d = F_last[:-1] + 2.0 * c_mid[1:] + B_row[1:]
    ans = min(cand_v.min(), cand_d.min())
    return np.float32(ans)



# revision 46
# speedup vs baseline: 1.0853x; 1.0853x over previous
"""DTW loss on trn2 — skewed-wavefront 2-core kernel, raw-bass phase B.

Same algorithm as kernel.py (skewed barber-pole wavefront, PE V-shift),
but emitted without TileContext: engines synchronize via manual counting
semaphores, so same-engine dependent ops run back-to-back without the
~95ns semaphore-propagation gap Tile inserts on every RAW hop.
"""

import sys

sys.path.insert(0, "/opt/trn_rl_repo")

from contextlib import ExitStack

import numpy as np

N = 4096
M = 4096
DIM = 64
N_ROWS = N // 2
F = 32
S = M // F
ROWS_PAD = N_ROWS + 256
NSTEP = N_ROWS + S - 1    # 2175
BIG = 1e30
ND = 12                   # dext rotation depth (capture WAR slack)
NCB = 6                   # cbuf rotation depth (cost-tile prefetch slack)

_nc_cache = {}


def _build_nc():
    if "nc" in _nc_cache:
        return _nc_cache["nc"]
    import concourse.bacc as bacc
    import concourse.bass as bass
    import concourse.mybir as mybir

    F32 = mybir.dt.float32
    BF16 = mybir.dt.bfloat16
    AluOp = mybir.AluOpType
    n_bands = N_ROWS // 128        # 16
    ROW_PITCH = 3 * M
    NB = 3 * n_bands               # 48 band-passes
    NMM = 8 * NB                   # 384 matmuls
    TB = 8
    NBATCH = (NSTEP + TB - 1) // TB

    nc = bacc.Bacc(None, target_bir_lowering=False)

    xt_d = nc.dram_tensor("xt", [65, N_ROWS], BF16, kind="ExternalInput")
    xst_d = nc.dram_tensor("xst", [65, N_ROWS], BF16, kind="ExternalInput")
    yt_d = nc.dram_tensor("yt", [65, M], BF16, kind="ExternalInput")
    yst_d = nc.dram_tensor("yst", [65, M], BF16, kind="ExternalInput")
    xx_d = nc.dram_tensor("xx", [128, N_ROWS // 128], F32, kind="ExternalInput")
    xxs_d = nc.dram_tensor("xxs", [128, N_ROWS // 128], F32, kind="ExternalInput")
    sh_d = nc.dram_tensor("sh", [128, 128], F32, kind="ExternalInput")
    idm_d = nc.dram_tensor("idm", [128, 128], F32, kind="ExternalInput")
    m0_d = nc.dram_tensor("m0", [128, 1], F32, kind="ExternalInput")
    dinit_d = nc.dram_tensor("dinit", [128, 33], F32, kind="ExternalInput")
    out_d = nc.dram_tensor("out", [M], F32, kind="ExternalOutput")
    SI = nc.dram_tensor("SI", [ROWS_PAD * ROW_PITCH], BF16)

    es = ExitStack()

    def sb(nm, shape, dt):
        return es.enter_context(nc.sbuf_tensor(nm, shape, dt)).ap()

    def psb(nm, shape):
        return es.enter_context(nc.psum_tensor(nm, shape, F32)).ap()

    xt_sb = sb("xt_sb", [65, N_ROWS], BF16)
    xst_sb = sb("xst_sb", [65, N_ROWS], BF16)
    yt_sb = sb("yt_sb", [65, M], BF16)
    yst_sb = sb("yst_sb", [65, M], BF16)
    xx_sb = sb("xx_sb", [128, n_bands], F32)
    xxs_sb = sb("xxs_sb", [128, n_bands], F32)
    sh_sb = sb("sh_sb", [128, 128], F32)
    m0 = sb("m0_sb", [128, 1], F32)
    dext = [sb(f"dext{i}", [128, F + 1], F32) for i in range(ND)]
    capline = sb("capline", [1, M // F * F], F32)
    idm_sb = sb("idm_sb", [128, 128], F32)
    zpad = sb("zpad", [128, 4096], BF16)
    c_band = [sb(f"c_band{i}", [128, M], BF16) for i in range(4)]
    cbuf = [sb(f"cbuf{i}", [128, 3 * TB * F], BF16) for i in range(NCB)]
    bigrow = sb("bigrow", [1, 128], F32)
    onec = sb("onec", [1, 1], F32)
    t1s = sb("t1s", [128, F], F32)
    t2s = sb("t2s", [128, F], F32)
    NPA = 2
    ps_a = [psb(f"ps_a{i}", [128, 512]) for i in range(NPA)]
    vps_all = psb("vps_all", [128, 4])
    vps = [vps_all[:, i:i + 1] for i in range(4)]
    ps_cap = [psb(f"ps_cap{i}", [1, 512]) for i in range(2)]

    s_in = nc.alloc_semaphore("s_in")
    s_z = nc.alloc_semaphore("s_z")
    s_capa = nc.alloc_semaphore("s_capa")
    s_flush = nc.alloc_semaphore("s_flush")
    s_mm = nc.alloc_semaphore("s_mm")
    s_act = nc.alloc_semaphore("s_act")
    s_store = nc.alloc_semaphore("s_store")
    s_pad = nc.alloc_semaphore("s_pad")
    s_cload = nc.alloc_semaphore("s_cload")
    s_dve = nc.alloc_semaphore("s_dve")
    s_shift = nc.alloc_semaphore("s_shift")
    s_cap = nc.alloc_semaphore("s_cap")

    IN_TOTAL = 10 * 16 + 1

    passes = (
        (xst_sb, xxs_sb, yt_sb),
        (xst_sb, xxs_sb, yst_sb),
        (xt_sb, xx_sb, yst_sb),
    )

    # ---------------- Pool: constants ---------------------------------
    nc.gpsimd.memset(zpad, 0.0).then_inc(s_z, 1)
    nc.gpsimd.memset(bigrow, 0.0)
    nc.gpsimd.memset(bigrow[0:1, 127:128], BIG)
    nc.gpsimd.memset(onec, 1.0).then_inc(s_in, 1)

    # ---------------- SP queue ----------------
    nc.sync.dma_start(xt_sb, xt_d[:]).then_inc(s_in, 16)
    nc.sync.dma_start(xst_sb, xst_d[:]).then_inc(s_in, 16)
    nc.sync.dma_start(yt_sb, yt_d[:]).then_inc(s_in, 16)
    nc.sync.dma_start(yst_sb, yst_d[:]).then_inc(s_in, 16)
    nc.sync.dma_start(xx_sb, xx_d[:]).then_inc(s_in, 16)
    nc.sync.dma_start(xxs_sb, xxs_d[:]).then_inc(s_in, 16)
    nc.sync.dma_start(sh_sb, sh_d[:]).then_inc(s_in, 16)
    nc.sync.dma_start(idm_sb, idm_d[:]).then_inc(s_in, 16)
    nc.sync.dma_start(m0, m0_d[:]).then_inc(s_in, 16)
    nc.sync.dma_start(dext[ND - 1], dinit_d[:]).then_inc(s_in, 16)
    # pad stores: gated only on the zpad memset, not on input landing
    nc.sync.wait_ge(s_z, 1)
    for r0 in (0, 128 + N_ROWS):
        for q in range(3):
            dst = bass.AP(
                SI[:].tensor, r0 * ROW_PITCH + q * M,
                [[ROW_PITCH, 128], [1, M]],
            )
            nc.sync.dma_start(dst, zpad).then_inc(s_pad, 16)
    PAD_HALF = 16 * 3
    for k in range(NBATCH):
        t0 = k * TB
        band_needed = min(n_bands - 1, (t0 + TB - 1) // 128)
        nc.sync.wait_ge(s_store, 16 * 3 * (band_needed + 1))
        if k >= NCB:
            nc.sync.wait_ge(s_dve, 8 * (k - NCB) + 8)
        if k == 0:
            nc.sync.wait_ge(s_pad, PAD_HALF)
        if k == 256:
            nc.sync.wait_ge(s_pad, 2 * PAD_HALF)
        for q in range(3):
            src = bass.AP(
                SI[:].tensor,
                (t0 + 1) * ROW_PITCH + q * M + 127 * F,
                [[ROW_PITCH - F, 128], [ROW_PITCH, TB], [1, F]],
            )
            nc.sync.dma_start(
                cbuf[k % NCB][:, q * TB * F:(q + 1) * TB * F], src
            ).then_inc(s_cload, 16)
    # single flush of the staged capture line
    nc.sync.wait_ge(s_capa, S // 16)
    nc.sync.dma_start(out_d[:], capline).then_inc(s_flush, 16)
    nc.sync.wait_ge(s_flush, 16)
    _epilogue_sems = True  # placed after all queues are emitted (see below)

    # ---------------- ACT queue ----------------
    nc.scalar.wait_ge(s_in, IN_TOTAL)
    for u in range(NB):
        b, q = divmod(u, 3)
        bias_sb = passes[q][1]
        cb_t = c_band[u % 4]
        if u >= 4:
            nc.scalar.wait_ge(s_store, 16 * (u - 3))
        for i in range(8):
            nc.scalar.wait_ge(s_mm, 8 * u + i + 1)
            nc.scalar.activation(
                cb_t[:, i * 512:(i + 1) * 512],
                ps_a[(8 * u + i) % NPA],
                mybir.ActivationFunctionType.Sqrt,
                bias=bias_sb[:, b:b + 1],
                scale=-2.0,
            ).then_inc(s_act, 1)
        dst = bass.AP(
            SI[:].tensor, (128 + b * 128) * ROW_PITCH + q * M,
            [[ROW_PITCH, 128], [1, M]],
        )
        nc.scalar.dma_start(dst, cb_t).then_inc(s_store, 16)
    # drain capture psum groups into the contiguous capture line
    for g in range(S // 16):
        nc.scalar.wait_ge(s_cap, 16 * (g + 1))
        nc.scalar.copy(
            capline[0:1, g * 512:(g + 1) * 512], ps_cap[g % 2]
        ).then_inc(s_capa, 1)

    # ---------------- PE queue ----------------
    def emit_mm(j):
        u, i = divmod(j, 8)
        b, q = divmod(u, 3)
        lhs, _, rhs = passes[q]
        if j >= NPA:
            nc.tensor.wait_ge(s_act, j - NPA + 1)
        nc.tensor.matmul(
            ps_a[j % NPA],
            lhs[:, b * 128:(b + 1) * 128],
            rhs[:, i * 512:(i + 1) * 512],
        ).then_inc(s_mm, 1)

    def emit_capture(tc):
        # row tc-(N_ROWS-1) of the final DP row: select partition
        # 127-s_c of dext[tc] via a unit-column matmul into the capture
        # psum line. PE-queue order (s_dve >= tc+1 held by the caller)
        # makes this race-free; dext[tc%ND] is reused ND steps later.
        s_c = tc - (N_ROWS - 1)
        g, slot = divmod(s_c, 16)
        mm = nc.tensor.matmul(
            ps_cap[g % 2][0:1, slot * F:(slot + 1) * F],
            idm_sb[:, 127 - s_c:128 - s_c],
            dext[tc % ND][:, 1:F + 1],
        )
        if slot == 15:
            mm.then_inc(s_cap, 16)

    nc.tensor.wait_ge(s_in, IN_TOTAL)
    for j in range(2 * 24):          # bands 0,1 upfront
        emit_mm(j)
    for t in range(NSTEP):
        if t >= 1:
            nc.tensor.wait_ge(s_dve, t)
        nc.tensor.matmul(
            vps[t % 4], sh_sb, dext[(t - 1) % ND][:, F:F + 1],
            start=True, stop=False,
        )
        nc.tensor.matmul(
            vps[t % 4], bigrow, onec, start=False, stop=True,
        ).then_inc(s_shift, 1)
        # drip-feed band (t//128 + 2)'s matmuls
        b = t // 128 + 2
        if b < n_bands:
            lt = t % 128
            if lt % 5 == 0 and lt // 5 < 24:
                emit_mm(24 * b + lt // 5)
        if N_ROWS - 1 <= t - 1 < N_ROWS - 1 + S:
            sc_prev = (t - 1) - (N_ROWS - 1)
            if sc_prev % 16 == 0 and sc_prev >= 32:
                # capture psum double-buffer WAR vs the ACT drain
                nc.tensor.wait_ge(s_capa, sc_prev // 16 - 1)
            emit_capture(t - 1)
    nc.tensor.wait_ge(s_dve, NSTEP)
    emit_capture(NSTEP - 1)

    # ---------------- DVE queue ----------------
    # NOTE: no engine-local ordering sem on the cross-step RAW edge —
    # same-engine program order covers it in this execution environment
    # (bass2jax/PJRT in-order queues); saves ~35ns/step of sem latency.
    nc.vector.wait_ge(s_in, IN_TOTAL)
    for t in range(NSTEP):
        k, dt = divmod(t, TB)
        if dt == 0:
            nc.vector.wait_ge(s_cload, 16 * 3 * (k + 1))
        cb = cbuf[k % NCB]
        a_t = cb[:, dt * F:(dt + 1) * F]
        b2_t = cb[:, (TB + dt) * F:(TB + dt + 1) * F]
        g_t = cb[:, (2 * TB + dt) * F:(2 * TB + dt + 1) * F]
        dcur = dext[t % ND]
        dprev = dext[(t - 1) % ND]
        nc.vector.tensor_tensor(t1s, dprev[:, 1:F + 1], a_t, op=AluOp.add)
        nc.vector.scalar_tensor_tensor(
            t2s, b2_t, 2.0, dprev[:, 0:F], op0=AluOp.mult, op1=AluOp.add
        )
        nc.vector.tensor_tensor(t2s, t1s, t2s, op=AluOp.min)
        nc.vector.wait_ge(s_shift, t + 1)
        # capture WAR on dext[t%ND] is covered by the s_shift chain: the
        # capture matmul for step t-ND ran before PE's shift for step t
        nc.vector.tensor_tensor_scan(
            dcur[:, 1:F + 1], g_t, t2s, vps[t % 4][:, 0:1],
            op0=AluOp.add, op1=AluOp.min,
        )
        # halo (next step's t2 shifted-D col 0) — off the critical path
        nc.vector.tensor_copy(dcur[:, 0:1], vps[t % 4][:, 0:1]).then_inc(
            s_dve, 1
        )

    # Tile-style epilogue: barrier + sem clear so the program is
    # re-executable (the runtime may run the NEFF more than once; stale
    # semaphore values would instantly satisfy every wait on rerun).
    nc.all_engine_barrier()
    nc.clear_and_free_semaphores(
        [s_in, s_z, s_mm, s_act, s_store, s_pad, s_cload, s_dve, s_shift,
         s_cap, s_capa, s_flush]
    )
    nc.all_engine_barrier()

    nc.compile()
    _nc_cache["nc"] = nc
    _nc_cache["es"] = es
    return nc


def _aug_t(a):
    """[n, 64] -> [65, n] transposed + ones row (bf16)."""
    import ml_dtypes
    n = a.shape[0]
    t = np.ones((65, n), np.float32)
    t[:64] = a.T
    return t.astype(ml_dtypes.bfloat16)


def _aug_y(yf):
    """[m, 64] -> [65, m]: y^T with row 64 = -0.5*(|y|^2 + 1e-12) (bf16)."""
    import ml_dtypes
    t = np.empty((65, yf.shape[0]), np.float32)
    t[:64] = yf.T
    yy = (yf.astype(np.float64) ** 2).sum(1)
    t[64] = (-0.5 * (yy + 1e-12)).astype(np.float32)
    return t.astype(ml_dtypes.bfloat16)


def _shift_mat():
    # partition p owns seg 127-p; V for seg s comes from seg s-1, i.e.
    # partition p+1: out[p] = rhs[p+1] -> lhsT[k,p]=1 iff k=p+1
    sh = np.zeros((128, 128), np.float32)
    for p in range(0, 127):
        sh[p + 1, p] = 1.0
    return sh


def _host_prep(xh, yf, core):
    if core == 0:
        xs, ys = xh, yf
    else:
        xs = np.concatenate([np.zeros((1, DIM), np.float32), xh[:-1]])
        # ys[0] pairs with x[0] to make C3[0,0] ~ 0 (H[0,0]=0). Perturb it
        # so the fp32-cancelled squared distance stays safely positive;
        # the 1.0 offset costs ~1e-5 relative error on the final answer.
        y0 = xh[0:1].copy()
        y0[0, 0] += 1.0
        ys = np.concatenate([y0, yf[:-1]])
    xx = (xh.astype(np.float64) ** 2).sum(1).astype(np.float32)
    xxs = (xs.astype(np.float64) ** 2).sum(1).astype(np.float32)
    return {
        "xt": _aug_t(xh),
        "xst": _aug_t(xs),
        "yt": _aug_y(yf),
        "yst": _aug_y(ys),
        # [p, b] = value for row b*128+p (contiguous [128, n_bands] load)
        "xx": np.ascontiguousarray(xx.reshape(-1, 128).T),
        "xxs": np.ascontiguousarray(xxs.reshape(-1, 128).T),
        "sh": _shift_mat(),
        "idm": np.eye(128, dtype=np.float32),
        "m0": _m0_mask(),
        "dinit": _dinit(),
    }


def _m0_mask():
    m = np.full((128, 1), -BIG, np.float32)
    m[127, 0] = BIG  # partition 127 = segment 0: V is always BIG there
    return m


def _dinit():
    d = np.full((128, 33), BIG, np.float32)
    d[127, 1] = 0.0  # virtual row -1: D(-1, global col 0) = 0
    return d




def kernel(x, y):
    x = np.ascontiguousarray(np.asarray(x, dtype=np.float32))
    y = np.ascontiguousarray(np.asarray(y, dtype=np.float32))
    assert x.shape == (N, DIM) and y.shape == (M, DIM)

    from concourse.bass_utils import run_bass_kernel_spmd

    nc = _build_nc()
    in_maps = [
        _host_prep(x[:N_ROWS], y, core=0),
        _host_prep(x[::-1][:N_ROWS].copy(), y[::-1].copy(), core=1),
    ]
    res = run_bass_kernel_spmd(nc, in_maps, core_ids=[0, 1])
    F_last = res.results[0]["out"].astype(np.float64)
    H_last = res.results[1]["out"].astype(np.float64)

    xm = x[N_ROWS].astype(np.float64)
    sq = (xm * xm).sum() + (y.astype(np.float64) ** 2).sum(1) - 2.0 * (
        y.astype(np.float64) @ xm
    )
    c_mid = np.sqrt(np.maximum(sq, 1e-12))
    B_row = H_last[::-1]
    cand_v = F_last + c_mid + B_row
    cand_d = F_last[:-1] + 2.0 * c_mid[1:] + B_row[1:]
    ans = min(cand_v.min(), cand_d.min())
    return np.float32(ans)

